# revision 6
# baseline (speedup 1.0000x reference)
"""Trainium2 Bass kernel: depthwise (per-sample, per-channel) 15x15 'same'
true convolution of 1024x3 images of 128x128, data-parallel over 8 NeuronCores.

Flipped-operand formulation (per (bn,c) pair, P=128, K=15, pad=7):
    out[y, x] = sum_{dy,dx} Xpad[y+dy, x+dx] * W[dy,dx],  W = flip(kernel)
y is split into 2 blocks of 64 rows. For block y0 and each dx:
    outT[x, y0+j] += sum_i Xpad[y0+i, x+dx] * T[i, dx, j]
with the banded block-Toeplitz T[i, dx, j] = W[i-j, dx] (0 <= i-j < 15).
The stationary operand is the padded-image slice Xpad[y0:y0+78, dx:dx+128]
(plain AP view); the moving operand is the compact Toeplitz streaming N=64
columns. PSUM holds the transposed output [x=128, y], accumulated over dx.

DMA economics dominate this kernel: DGE descriptor generation costs ~7.5ns
per packet on the issuing queue, so inputs/outputs are batched 8 images per
DMA with packet-major DRAM layouts ([row, img, col] etc.) giving one long
contiguous run per partition per group. X/out DMAs issue on the sync queue,
toep on the scalar (Activation) queue. Output fp16, transposed; the host
transposes back and upcasts. Sharding: pure data parallel over BN (384
independent images per core).
"""
import sys

sys.path.insert(0, "/opt/trn_rl_repo")

import numpy as np

_N_CORES = 8
_BN, _C, _P, _K = 1024, 3, 128, 15
_PAIRS_PER_CORE = (_BN // _N_CORES) * _C  # 384
_GROUP = 8

_nc_cache = {}


def _build_nc(n_pairs: int, bufs: int = 3, psum_bufs: int = 4):
    import concourse.bacc as bacc
    import concourse.mybir as mybir
    from concourse import tile

    FP16 = mybir.dt.float16
    FP32 = mybir.dt.float32
    G = _GROUP
    n_groups = n_pairs // G

    nc = bacc.Bacc("TRN2", target_bir_lowering=False, debug=False)
    xpad_d = nc.dram_tensor("xpad", [142, n_pairs, 142], FP16, kind="ExternalInput")
    toep_d = nc.dram_tensor("toep", [78, n_pairs, 960], FP16, kind="ExternalInput")
    # block A (y 0..63) classic orientation: outA[y, img, x]
    # block B (y 64..127) flipped orientation: outB[x, img, y-64]
    outa_d = nc.dram_tensor("outa", [64, n_pairs, 128], FP16, kind="ExternalOutput")
    outb_d = nc.dram_tensor("outb", [128, n_pairs, 64], FP16, kind="ExternalOutput")

    with tile.TileContext(nc) as tc:
        with (
            tc.tile_pool(name="xa", bufs=bufs) as xa_pool,
            tc.tile_pool(name="xb", bufs=bufs) as xb_pool,
            tc.tile_pool(name="tt", bufs=bufs) as tt_pool,
            tc.tile_pool(name="ot", bufs=bufs) as ot_pool,
            tc.tile_pool(name="ps", bufs=psum_bufs, space="PSUM") as ps_pool,
        ):
            for g in range(n_groups):
                s = slice(g * G, (g + 1) * G)
                xa = xa_pool.tile([78, G * 142], FP16, tag="xa")
                xb = xb_pool.tile([78, G * 142], FP16, tag="xb")
                tt = tt_pool.tile([78, G * 960], FP16, tag="tt")
                ota = ot_pool.tile([64, G * 128], FP16, tag="ota")
                otb = ot_pool.tile([128, G * 64], FP16, tag="otb")
                nc.sync.dma_start(out=xa[:], in_=xpad_d[0:78, s, :])
                nc.sync.dma_start(out=xb[:], in_=xpad_d[64:142, s, :])
                nc.scalar.dma_start(out=tt[:], in_=toep_d[:, s, :])

                for j in range(G):
                    psa = ps_pool.tile([64, 128], FP32, tag="psa")
                    psb = ps_pool.tile([128, 64], FP32, tag="psb")
                    xo = j * 142
                    to = j * 960
                    for dx in range(15):
                        tslc = tt[:, to + dx * 64 : to + (dx + 1) * 64]
                        # classic: stationary = Toeplitz (load 64), stream xa
                        # columns (128); next flipped load (128) hides here
                        nc.tensor.matmul(
                            psa[:], tslc, xa[:, xo + dx : xo + dx + 128],
                            start=(dx == 0), stop=(dx == 14),
                        )
                        # flipped: stationary = xb slice (load 128), stream
                        # Toeplitz columns (64); next classic load (64) hides
                        nc.tensor.matmul(
                            psb[:], xb[:, xo + dx : xo + dx + 128], tslc,
                            start=(dx == 0), stop=(dx == 14),
                        )
                    nc.vector.tensor_copy(ota[:, j * 128 : (j + 1) * 128], psa[:])
                    nc.scalar.copy(otb[:, j * 64 : (j + 1) * 64], psb[:])
                nc.sync.dma_start(out=outa_d[:, s, :], in_=ota[:])
                nc.scalar.dma_start(out=outb_d[:, s, :], in_=otb[:])

    nc.compile()
    return nc


def _host_prep(patches_pairs: np.ndarray, kernels_pairs: np.ndarray):
    """[NP,128,128] f32, [NP,15,15] f32 -> xpad [142,NP,142] fp16 (row-major
    over images), toep [78,NP,960] fp16 with toep[i,p,dx*64+j] =
    flip(kern)[i-j, dx]."""
    NP = patches_pairs.shape[0]
    Xp = np.zeros((NP, 142, 142), dtype=np.float16)
    Xp[:, 7:135, 7:135] = patches_pairs.astype(np.float16)
    Xp = np.ascontiguousarray(Xp.transpose(1, 0, 2))
    W = kernels_pairs[:, ::-1, ::-1].astype(np.float16)
    H = np.zeros((NP, 141, 15), dtype=np.float16)
    H[:, 63:78, :] = W
    s0, s1, s2 = H.strides
    A = np.lib.stride_tricks.as_strided(
        H[:, 63:, :], shape=(NP, 78, 64, 15), strides=(s0, s1, -s1, s2)
    )
    T = np.ascontiguousarray(
        A.transpose(1, 0, 3, 2).reshape(78, NP, 960)
    )
    return Xp, T


def kernel(patches, kernels, kernel_size, patch_size, fft_size, _collect_results=None):
    """Full inputs in, full output out. Shards BN across 8 cores."""
    from concourse.bass_utils import run_bass_kernel_spmd

    patches = np.asarray(patches)
    kernels = np.asarray(kernels)
    assert patches.shape == (_BN, _C, _P, _P), patches.shape
    assert kernels.shape == (_BN, _C, _K, _K), kernels.shape

    if "nc" not in _nc_cache:
        _nc_cache["nc"] = _build_nc(_PAIRS_PER_CORE)
    nc = _nc_cache["nc"]

    bn_per_core = _BN // _N_CORES
    in_maps = []
    for core in range(_N_CORES):
        sl = slice(core * bn_per_core, (core + 1) * bn_per_core)
        pp = patches[sl].reshape(-1, _P, _P)
        kp = kernels[sl].reshape(-1, _K, _K)
        xpad, toep = _host_prep(pp, kp)
        in_maps.append({"xpad": xpad, "toep": toep})

    res = run_bass_kernel_spmd(nc, in_maps, core_ids=list(range(_N_CORES)))
    if _collect_results is not None:
        _collect_results.append(res)

    out = np.empty((_BN, _C, _P, _P), dtype=np.float32)
    for core in range(_N_CORES):
        sl = slice(core * bn_per_core, (core + 1) * bn_per_core)
        # block A arrives [y, pair, x]; block B arrives transposed [x, pair, y]
        oa = res.results[core]["outa"].astype(np.float32).transpose(1, 0, 2)
        ob = res.results[core]["outb"].astype(np.float32).transpose(1, 2, 0)
        full = np.concatenate([oa, ob], axis=1)  # [pairs, 128, 128]
        out[sl] = full.reshape(bn_per_core, _C, _P, _P)
    return out


# revision 8
# speedup vs baseline: 1.0004x; 1.0004x over previous
"""Trainium2 Bass kernel: depthwise (per-sample, per-channel) 15x15 'same'
true convolution of 1024x3 images of 128x128, data-parallel over 8 NeuronCores.

Flipped-operand formulation (per (bn,c) pair, P=128, K=15, pad=7):
    out[y, x] = sum_{dy,dx} Xpad[y+dy, x+dx] * W[dy,dx],  W = flip(kernel)
y is split into 2 blocks of 64 rows. For block y0 and each dx:
    outT[x, y0+j] += sum_i Xpad[y0+i, x+dx] * T[i, dx, j]
with the banded block-Toeplitz T[i, dx, j] = W[i-j, dx] (0 <= i-j < 15).
The stationary operand is the padded-image slice Xpad[y0:y0+78, dx:dx+128]
(plain AP view); the moving operand is the compact Toeplitz streaming N=64
columns. PSUM holds the transposed output [x=128, y], accumulated over dx.

DMA economics dominate this kernel: DGE descriptor generation costs ~7.5ns
per packet on the issuing queue, so inputs/outputs are batched 8 images per
DMA with packet-major DRAM layouts ([row, img, col] etc.) giving one long
contiguous run per partition per group. X/out DMAs issue on the sync queue,
toep on the scalar (Activation) queue. Output fp16, transposed; the host
transposes back and upcasts. Sharding: pure data parallel over BN (384
independent images per core).
"""
import sys

sys.path.insert(0, "/opt/trn_rl_repo")

import numpy as np

_N_CORES = 8
_BN, _C, _P, _K = 1024, 3, 128, 15
_PAIRS_PER_CORE = (_BN // _N_CORES) * _C  # 384
_GROUP = 8

_nc_cache = {}


def _build_nc(n_pairs: int, bufs: int = 3, psum_bufs: int = 4):
    import concourse.bacc as bacc
    import concourse.mybir as mybir
    from concourse import tile

    FP16 = mybir.dt.float16
    FP32 = mybir.dt.float32
    G = _GROUP
    n_groups = n_pairs // G

    nc = bacc.Bacc("TRN2", target_bir_lowering=False, debug=False)
    xpad_d = nc.dram_tensor("xpad", [142, n_pairs, 142], FP16, kind="ExternalInput")
    toep_d = nc.dram_tensor("toep", [78, n_pairs, 960], FP16, kind="ExternalInput")
    # block A (y 0..63) classic orientation: outA[y, img, x]
    # block B (y 64..127) flipped orientation: outB[x, img, y-64]
    outa_d = nc.dram_tensor("outa", [64, n_pairs, 128], FP16, kind="ExternalOutput")
    outb_d = nc.dram_tensor("outb", [128, n_pairs, 64], FP16, kind="ExternalOutput")

    with tile.TileContext(nc) as tc:
        with (
            tc.tile_pool(name="xa", bufs=bufs) as xa_pool,
            tc.tile_pool(name="xb", bufs=bufs) as xb_pool,
            tc.tile_pool(name="tt", bufs=bufs) as tt_pool,
            tc.tile_pool(name="ot", bufs=bufs) as ot_pool,
            tc.tile_pool(name="ps", bufs=psum_bufs, space="PSUM") as ps_pool,
        ):
            for g in range(n_groups):
                s = slice(g * G, (g + 1) * G)
                xa = xa_pool.tile([78, G * 142], FP16, tag="xa")
                xb = xb_pool.tile([78, G * 142], FP16, tag="xb")
                tt = tt_pool.tile([78, G * 960], FP16, tag="tt")
                ota = ot_pool.tile([64, G * 128], FP16, tag="ota")
                otb = ot_pool.tile([128, G * 64], FP16, tag="otb")
                nc.sync.dma_start(out=xa[:], in_=xpad_d[0:78, s, :])
                nc.sync.dma_start(out=xb[:], in_=xpad_d[64:142, s, :])
                nc.scalar.dma_start(out=tt[:], in_=toep_d[:, s, :])

                for j in range(G):
                    # full-bank tiles: avoid two accumulation groups
                    # sharing one PSUM bank
                    psa_t = ps_pool.tile([128, 512], FP32, tag="psa")
                    psb_t = ps_pool.tile([128, 512], FP32, tag="psb")
                    psa = psa_t[0:64, 0:128]
                    psb = psb_t[:, 0:64]
                    xo = j * 142
                    to = j * 960
                    for dx in range(15):
                        tslc = tt[:, to + dx * 64 : to + (dx + 1) * 64]
                        # classic: stationary = Toeplitz (load 64), stream xa
                        # columns (128); next flipped load (128) hides here
                        nc.tensor.matmul(
                            psa, tslc, xa[:, xo + dx : xo + dx + 128],
                            start=(dx == 0), stop=(dx == 14),
                        )
                        # flipped: stationary = xb slice (load 128), stream
                        # Toeplitz columns (64); next classic load (64) hides
                        nc.tensor.matmul(
                            psb, xb[:, xo + dx : xo + dx + 128], tslc,
                            start=(dx == 0), stop=(dx == 14),
                        )
                    nc.vector.tensor_copy(ota[:, j * 128 : (j + 1) * 128], psa)
                    nc.scalar.copy(otb[:, j * 64 : (j + 1) * 64], psb)
                nc.sync.dma_start(out=outa_d[:, s, :], in_=ota[:])
                nc.scalar.dma_start(out=outb_d[:, s, :], in_=otb[:])

    nc.compile()
    return nc


def _host_prep(patches_pairs: np.ndarray, kernels_pairs: np.ndarray):
    """[NP,128,128] f32, [NP,15,15] f32 -> xpad [142,NP,142] fp16 (row-major
    over images), toep [78,NP,960] fp16 with toep[i,p,dx*64+j] =
    flip(kern)[i-j, dx]."""
    NP = patches_pairs.shape[0]
    Xp = np.zeros((NP, 142, 142), dtype=np.float16)
    Xp[:, 7:135, 7:135] = patches_pairs.astype(np.float16)
    Xp = np.ascontiguousarray(Xp.transpose(1, 0, 2))
    W = kernels_pairs[:, ::-1, ::-1].astype(np.float16)
    H = np.zeros((NP, 141, 15), dtype=np.float16)
    H[:, 63:78, :] = W
    s0, s1, s2 = H.strides
    A = np.lib.stride_tricks.as_strided(
        H[:, 63:, :], shape=(NP, 78, 64, 15), strides=(s0, s1, -s1, s2)
    )
    T = np.ascontiguousarray(
        A.transpose(1, 0, 3, 2).reshape(78, NP, 960)
    )
    return Xp, T


def kernel(patches, kernels, kernel_size, patch_size, fft_size, _collect_results=None):
    """Full inputs in, full output out. Shards BN across 8 cores."""
    from concourse.bass_utils import run_bass_kernel_spmd

    patches = np.asarray(patches)
    kernels = np.asarray(kernels)
    assert patches.shape == (_BN, _C, _P, _P), patches.shape
    assert kernels.shape == (_BN, _C, _K, _K), kernels.shape

    if "nc" not in _nc_cache:
        _nc_cache["nc"] = _build_nc(_PAIRS_PER_CORE)
    nc = _nc_cache["nc"]

    bn_per_core = _BN // _N_CORES
    in_maps = []
    for core in range(_N_CORES):
        sl = slice(core * bn_per_core, (core + 1) * bn_per_core)
        pp = patches[sl].reshape(-1, _P, _P)
        kp = kernels[sl].reshape(-1, _K, _K)
        xpad, toep = _host_prep(pp, kp)
        in_maps.append({"xpad": xpad, "toep": toep})

    res = run_bass_kernel_spmd(nc, in_maps, core_ids=list(range(_N_CORES)))
    if _collect_results is not None:
        _collect_results.append(res)

    out = np.empty((_BN, _C, _P, _P), dtype=np.float32)
    for core in range(_N_CORES):
        sl = slice(core * bn_per_core, (core + 1) * bn_per_core)
        # block A arrives [y, pair, x]; block B arrives transposed [x, pair, y]
        oa = res.results[core]["outa"].astype(np.float32).transpose(1, 0, 2)
        ob = res.results[core]["outb"].astype(np.float32).transpose(1, 2, 0)
        full = np.concatenate([oa, ob], axis=1)  # [pairs, 128, 128]
        out[sl] = full.reshape(bn_per_core, _C, _P, _P)
    return out


# revision 9
# speedup vs baseline: 2.8893x; 2.8881x over previous
"""Trainium2 Bass kernel: depthwise (per-sample, per-channel) 15x15 'same'
true convolution of 1024x3 images of 128x128, data-parallel over 8 NeuronCores.

Flipped-operand formulation (per (bn,c) pair, P=128, K=15, pad=7):
    out[y, x] = sum_{dy,dx} Xpad[y+dy, x+dx] * W[dy,dx],  W = flip(kernel)
y is split into 2 blocks of 64 rows. For block y0 and each dx:
    outT[x, y0+j] += sum_i Xpad[y0+i, x+dx] * T[i, dx, j]
with the banded block-Toeplitz T[i, dx, j] = W[i-j, dx] (0 <= i-j < 15).
The stationary operand is the padded-image slice Xpad[y0:y0+78, dx:dx+128]
(plain AP view); the moving operand is the compact Toeplitz streaming N=64
columns. PSUM holds the transposed output [x=128, y], accumulated over dx.

DMA economics dominate this kernel: DGE descriptor generation costs ~7.5ns
per packet on the issuing queue, so inputs/outputs are batched 8 images per
DMA with packet-major DRAM layouts ([row, img, col] etc.) giving one long
contiguous run per partition per group. X/out DMAs issue on the sync queue,
toep on the scalar (Activation) queue. Output fp16, transposed; the host
transposes back and upcasts. Sharding: pure data parallel over BN (384
independent images per core).
"""
import sys

sys.path.insert(0, "/opt/trn_rl_repo")

import numpy as np

_N_CORES = 8
_BN, _C, _P, _K = 1024, 3, 128, 15
_PAIRS_PER_CORE = (_BN // _N_CORES) * _C  # 384
_GROUP = 8

_nc_cache = {}


def _build_nc(n_pairs: int, bufs: int = 4, psum_bufs: int = 4):
    import concourse.bacc as bacc
    import concourse.mybir as mybir
    from concourse import tile

    FP16 = mybir.dt.float16
    FP32 = mybir.dt.float32
    G = _GROUP
    n_groups = n_pairs // G

    nc = bacc.Bacc("TRN2", target_bir_lowering=False, debug=False)
    xpad_d = nc.dram_tensor("xpad", [142, n_pairs, 142], FP16, kind="ExternalInput")
    toep_d = nc.dram_tensor("toep", [78, n_pairs, 960], FP16, kind="ExternalInput")
    out_d = nc.dram_tensor("out", [128, n_pairs, 128], FP16, kind="ExternalOutput")

    with tile.TileContext(nc) as tc:
        with (
            tc.tile_pool(name="xa", bufs=bufs) as xa_pool,
            tc.tile_pool(name="xb", bufs=bufs) as xb_pool,
            tc.tile_pool(name="tt", bufs=bufs) as tt_pool,
            tc.tile_pool(name="ot", bufs=bufs) as ot_pool,
            tc.tile_pool(name="ps", bufs=psum_bufs, space="PSUM") as ps_pool,
        ):
            for g in range(n_groups):
                s = slice(g * G, (g + 1) * G)
                xa = xa_pool.tile([78, G * 142], FP16, tag="xa")
                xb = xb_pool.tile([78, G * 142], FP16, tag="xb")
                tt = tt_pool.tile([78, G * 960], FP16, tag="tt")
                ot = ot_pool.tile([128, G * 128], FP16, tag="ot")
                nc.sync.dma_start(out=xa[:], in_=xpad_d[0:78, s, :])
                nc.sync.dma_start(out=xb[:], in_=xpad_d[64:142, s, :])
                nc.scalar.dma_start(out=tt[:], in_=toep_d[:, s, :])

                for j in range(G):
                    ps0 = ps_pool.tile([128, 64], FP32, tag="ps0")
                    ps1 = ps_pool.tile([128, 64], FP32, tag="ps1")
                    xo = j * 142
                    to = j * 960
                    for dx in range(15):
                        tslc = tt[:, to + dx * 64 : to + (dx + 1) * 64]
                        nc.tensor.matmul(
                            ps0[:], xa[:, xo + dx : xo + dx + 128], tslc,
                            start=(dx == 0), stop=(dx == 14),
                        )
                        nc.tensor.matmul(
                            ps1[:], xb[:, xo + dx : xo + dx + 128], tslc,
                            start=(dx == 0), stop=(dx == 14),
                        )
                    oo = j * 128
                    nc.vector.tensor_copy(ot[:, oo : oo + 64], ps0[:])
                    nc.scalar.copy(ot[:, oo + 64 : oo + 128], ps1[:])
                nc.sync.dma_start(out=out_d[:, s, :], in_=ot[:])

    nc.compile()
    return nc


def _host_prep(patches_pairs: np.ndarray, kernels_pairs: np.ndarray):
    """[NP,128,128] f32, [NP,15,15] f32 -> xpad [142,NP,142] fp16 (row-major
    over images), toep [78,NP,960] fp16 with toep[i,p,dx*64+j] =
    flip(kern)[i-j, dx]."""
    NP = patches_pairs.shape[0]
    Xp = np.zeros((NP, 142, 142), dtype=np.float16)
    Xp[:, 7:135, 7:135] = patches_pairs.astype(np.float16)
    Xp = np.ascontiguousarray(Xp.transpose(1, 0, 2))
    W = kernels_pairs[:, ::-1, ::-1].astype(np.float16)
    H = np.zeros((NP, 141, 15), dtype=np.float16)
    H[:, 63:78, :] = W
    s0, s1, s2 = H.strides
    A = np.lib.stride_tricks.as_strided(
        H[:, 63:, :], shape=(NP, 78, 64, 15), strides=(s0, s1, -s1, s2)
    )
    T = np.ascontiguousarray(
        A.transpose(1, 0, 3, 2).reshape(78, NP, 960)
    )
    return Xp, T


def kernel(patches, kernels, kernel_size, patch_size, fft_size, _collect_results=None):
    """Full inputs in, full output out. Shards BN across 8 cores."""
    from concourse.bass_utils import run_bass_kernel_spmd

    patches = np.asarray(patches)
    kernels = np.asarray(kernels)
    assert patches.shape == (_BN, _C, _P, _P), patches.shape
    assert kernels.shape == (_BN, _C, _K, _K), kernels.shape

    if "nc" not in _nc_cache:
        _nc_cache["nc"] = _build_nc(_PAIRS_PER_CORE)
    nc = _nc_cache["nc"]

    bn_per_core = _BN // _N_CORES
    in_maps = []
    for core in range(_N_CORES):
        sl = slice(core * bn_per_core, (core + 1) * bn_per_core)
        pp = patches[sl].reshape(-1, _P, _P)
        kp = kernels[sl].reshape(-1, _K, _K)
        xpad, toep = _host_prep(pp, kp)
        in_maps.append({"xpad": xpad, "toep": toep})

    res = run_bass_kernel_spmd(nc, in_maps, core_ids=list(range(_N_CORES)))
    if _collect_results is not None:
        _collect_results.append(res)

    out = np.empty((_BN, _C, _P, _P), dtype=np.float32)
    for core in range(_N_CORES):
        sl = slice(core * bn_per_core, (core + 1) * bn_per_core)
        # device emits outT [x, pair, y] -> [pair, y, x]
        outT = res.results[core]["out"].astype(np.float32)
        out[sl] = outT.transpose(1, 2, 0).reshape(bn_per_core, _C, _P, _P)
    return out
